# revision 2
# baseline (speedup 1.0000x reference)
"""Masked (sparse) attention for Trainium2 — Bass/Tile kernel, 8 NeuronCores.

v4: mask streams HBM->SBUF as dense bf16 via SWDGE cast-DMA (fp32 0/1 casts
exactly).  Mask application is split three ways by group to balance engines:
  - PE path:   S += BIG*mask via identity-matmul pre-exp (exp bias -BIG/8)
  - DVE path:  P = exp(S/8) * mask post-exp (tensor_mul, 2x packed bf16)
  - Pool path: same multiply on the (otherwise idle) GpSimd engine
Softmax denominators ride along as an all-ones column appended to V (PV
accumulates sum_m P^T[m,q] into out_ps[:, dv]), so no activation accum_out
and no separate row-sum pass.  The per-nb group stream is software-pipelined
with a 2-group skew: PE emission order is S(g) / maskMM(g) / transposes(g-2)
/ copy / PV so the tensor engine never idles behind the exp->mult chain.
out_acc is double-buffered per iteration to decouple the output DMA from the
next iteration's head.
"""

import os
import sys

import numpy as np

for _p in ("/opt/trn_rl_repo", "/root/.axon_site/_ro/trn_rl_repo"):
    if os.path.isdir(_p) and _p not in sys.path:
        sys.path.append(_p)

from contextlib import ExitStack

import concourse.bass as bass  # noqa: E402
import concourse.mybir as mybir  # noqa: E402
import concourse.tile as tile  # noqa: E402
from concourse import bacc  # noqa: E402
from concourse.bass_utils import run_bass_kernel_spmd  # noqa: E402
from concourse.masks import make_identity  # noqa: E402

FP32 = mybir.dt.float32
BF16 = mybir.dt.bfloat16
FP16 = mybir.dt.float16
AF = mybir.ActivationFunctionType
ALU = mybir.AluOpType
AXL = mybir.AxisListType

BIG = 128.0
N_CORES = 8
B, N, M, D, DV = 8, 4096, 4096, 64, 64

# mask-path schedule per 8 groups: 'p'=PE identity-MM, 'd'=DVE mult,
# 'g'=GpSimd mult  (3:2:3 split)
MASK_SCHED = "dgdgddgd"


def _build(n=N, m=M, d=D, dv=DV, reps=1, n_cores=N_CORES, group=1024,
           mask_bufs=6, p_bufs=4, sched=MASK_SCHED, skew=2, s_chunk=512):
    lp = FP16          # Q/K dtype (S matmul precision)
    mp_ = BF16         # mask / P / V dtype
    NB = n // 128
    MB = m // 128
    NG = m // group
    GB = group // 128
    TOT = NB * NG      # total groups per iteration
    assert n % 128 == 0 and m % group == 0 and group % 128 == 0

    nc = bacc.Bacc("TRN2", target_bir_lowering=False, debug=False,
                   num_devices=n_cores)

    q_d = nc.dram_tensor("queries", [n, d], FP32, kind="ExternalInput").ap()
    k_d = nc.dram_tensor("keys", [m, d], FP32, kind="ExternalInput").ap()
    v_d = nc.dram_tensor("values", [m, dv], FP32, kind="ExternalInput").ap()
    mask_d = nc.dram_tensor("visible_masking", [n, m], FP32,
                            kind="ExternalInput").ap()
    out_d = nc.dram_tensor("out", [n, dv], FP32, kind="ExternalOutput").ap()

    q_dv = q_d.rearrange("(b p) x -> p b x", p=128)
    k_dv = k_d.rearrange("(b p) x -> p b x", p=128)
    v_dv = v_d.rearrange("(b p) x -> p b x", p=128)
    out_dv = out_d.rearrange("(b p) x -> p b x", p=128)

    with tile.TileContext(nc) as tc, ExitStack() as ctx:
        per = ctx.enter_context(tc.tile_pool(name="persist", bufs=1))
        ident_lp = per.tile([128, 128], mp_)
        make_identity(nc, ident_lp)
        big_i = per.tile([128, 128], mp_)
        make_identity(nc, big_i)
        nc.vector.tensor_scalar_mul(big_i[:], big_i[:], BIG)
        ident_f32 = per.tile([128, 128], FP32)
        make_identity(nc, ident_f32)
        exp_bias = per.tile([128, 1], FP32)
        nc.vector.memset(exp_bias[:], -BIG / 8.0)
        qt_sb = per.tile([d, n], lp)
        kt_sb = per.tile([d, m], lp)
        v_sb = per.tile([128, MB * (dv + 1)], mp_)

        # setup: batched loads, PE transposes for Q^T/K^T, V' build
        with tc.tile_pool(name="setup", bufs=2) as sp, \
             tc.tile_pool(name="setup_ps", bufs=4, space="PSUM") as spp:
            q_all = sp.tile([128, NB * d], FP32, tag="qk")
            nc.sync.dma_start(q_all[:].rearrange("p (b x) -> p b x", x=d),
                              q_dv)
            k_all = sp.tile([128, MB * d], FP32, tag="qk")
            nc.sync.dma_start(k_all[:].rearrange("p (b x) -> p b x", x=d),
                              k_dv)
            v_all = sp.tile([128, MB * dv], FP32, tag="qk")
            nc.sync.dma_start(v_all[:].rearrange("p (b x) -> p b x", x=dv),
                              v_dv)
            nc.vector.memset(v_sb[:], 1.0)
            nc.vector.tensor_copy(
                v_sb[:].rearrange("p (b x) -> p b x", x=dv + 1)[:, :, 0:dv],
                v_all[:].rearrange("p (b x) -> p b x", x=dv))
            for nb in range(NB):
                qt_ps = spp.tile([d, 128], FP32, tag="tp")
                nc.tensor.transpose(qt_ps[:], q_all[:, nb * d:(nb + 1) * d],
                                    ident_f32[:])
                nc.vector.tensor_copy(qt_sb[:, nb * 128:(nb + 1) * 128],
                                      qt_ps[:])
            for mb in range(MB):
                kt_ps = spp.tile([d, 128], FP32, tag="tp")
                nc.tensor.transpose(kt_ps[:], k_all[:, mb * d:(mb + 1) * d],
                                    ident_f32[:])
                nc.vector.tensor_copy(kt_sb[:, mb * 128:(mb + 1) * 128],
                                      kt_ps[:])

        mp = ctx.enter_context(tc.tile_pool(name="maskp", bufs=mask_bufs))
        bp = ctx.enter_context(tc.tile_pool(name="bfp", bufs=p_bufs))
        ptp = ctx.enter_context(tc.tile_pool(name="ptsbp", bufs=3))
        sps = ctx.enter_context(tc.tile_pool(name="spsum", bufs=2,
                                             space="PSUM"))
        pps = ctx.enter_context(tc.tile_pool(name="ptpsum", bufs=2,
                                             space="PSUM"))
        ops_ = ctx.enter_context(tc.tile_pool(name="opsum", bufs=2,
                                              space="PSUM"))
        fp = ctx.enter_context(tc.tile_pool(name="finp", bufs=2))
        oap = ctx.enter_context(tc.tile_pool(name="oaccp", bufs=2))

        for _ in range(reps):
            out_acc = oap.tile([128, NB * dv], FP32, tag="oacc")
            masks = {}      # nb -> (mask half-tiles)
            state = {}      # gg -> (p_sb tile, out_ps tile)
            state2 = {}     # gg -> (pt_sb tile, out_ps tile)
            ops_by_nb = {}

            def emit_front(gg):
                nb, g = divmod(gg, NG)
                no = nb * 128
                go = g * group
                path = sched[gg % len(sched)]
                if g == 0:
                    mask_t = mp.tile([128, m], mp_, tag="mask")
                    nc.gpsimd.dma_start(mask_t[:], mask_d[no:no + 128, :])
                    masks[nb] = mask_t
                    ops_by_nb[nb] = ops_.tile([128, dv + 1], FP32, tag="acc",
                                              name=f"out_ps_{nb}")
                half = masks[nb]
                ho = go
                s_ps = sps.tile([128, group], FP32, tag="s")
                for c in range(0, group, s_chunk):
                    nc.tensor.matmul(
                        s_ps[:, c:c + s_chunk],
                        lhsT=qt_sb[:, no:no + 128],
                        rhs=kt_sb[:, go + c:go + c + s_chunk],
                        start=True, stop=(path != "p"))
                if path == "p":
                    for c in range(0, group, s_chunk):
                        nc.tensor.matmul(
                            s_ps[:, c:c + s_chunk],
                            lhsT=big_i[:],
                            rhs=half[:, ho + c:ho + c + s_chunk],
                            start=False, stop=True)
                p_sb = bp.tile([128, group], mp_, tag="p")
                if path == "p":
                    nc.scalar.activation(p_sb[:], s_ps[:], AF.Exp,
                                         bias=exp_bias[:], scale=1.0 / 8.0)
                    pm_sb = p_sb
                else:
                    nc.scalar.activation(p_sb[:], s_ps[:], AF.Exp,
                                         scale=1.0 / 8.0)
                    eng = nc.vector if path == "d" else nc.gpsimd
                    pm_sb = bp.tile([128, group], mp_, tag="pm")
                    eng.tensor_mul(pm_sb[:], p_sb[:],
                                   half[:, ho:ho + group])
                state[gg] = (pm_sb, ops_by_nb[nb])

            def emit_back(gg):
                nb, g = divmod(gg, NG)
                p_sb, out_ps = state.pop(gg)
                pt_ps = pps.tile([128, group], mp_, tag="pt")
                for k in range(GB):
                    nc.tensor.transpose(
                        pt_ps[:, k * 128:(k + 1) * 128],
                        p_sb[:, k * 128:(k + 1) * 128],
                        ident_lp[:])
                pt_sb = ptp.tile([128, group], mp_, tag="ptsb")
                nc.vector.tensor_copy(pt_sb[:], pt_ps[:])
                for k in range(GB):
                    mb = g * GB + k
                    nc.tensor.matmul(
                        out_ps[:],
                        lhsT=pt_sb[:, k * 128:(k + 1) * 128],
                        rhs=v_sb[:, mb * (dv + 1):(mb + 1) * (dv + 1)],
                        start=(mb == 0), stop=(mb == MB - 1))
                if g == NG - 1:
                    ops_by_nb.pop(nb, None)
                    recip = fp.tile([128, 1], FP32, tag="recip")
                    nc.vector.reciprocal(recip[:], out_ps[:, dv:dv + 1])
                    nc.vector.tensor_scalar(
                        out_acc[:, nb * dv:(nb + 1) * dv],
                        out_ps[:, 0:dv], recip[:], None, ALU.mult)

            for gg in range(TOT + skew):
                if gg < TOT:
                    emit_front(gg)
                if gg >= skew:
                    emit_back(gg - skew)
            nc.sync.dma_start(out_dv,
                              out_acc[:].rearrange("p (b x) -> p b x", x=dv))

    nc.compile()
    return nc


_CACHE = {}


def _get_nc(reps=1):
    key = ("nc", reps)
    if key not in _CACHE:
        _CACHE[key] = _build(reps=reps)
    return _CACHE[key]


def kernel(queries, keys, values, visible_masking):
    """Full inputs [8, 4096, ...] -> full output [8, 4096, 64] (fp32)."""
    queries = np.ascontiguousarray(np.asarray(queries, dtype=np.float32))
    keys = np.ascontiguousarray(np.asarray(keys, dtype=np.float32))
    values = np.ascontiguousarray(np.asarray(values, dtype=np.float32))
    visible_masking = np.ascontiguousarray(
        np.asarray(visible_masking, dtype=np.float32))
    assert queries.shape == (B, N, D), queries.shape

    nc = _get_nc()
    in_maps = [{
        "queries": queries[c],
        "keys": keys[c],
        "values": values[c],
        "visible_masking": visible_masking[c],
    } for c in range(N_CORES)]
    res = run_bass_kernel_spmd(nc, in_maps, core_ids=list(range(N_CORES)))
    return np.stack([res.results[c]["out"] for c in range(N_CORES)], axis=0)
